# revision 17
# baseline (speedup 1.0000x reference)
"""Self-contained TRN2 Bass kernel for nn_FLoRALayer (B=8, S=2048, D=1024, R=8).

kernel(**inputs) takes FULL unsharded inputs:
    x         [8, 2048, 1024] f32
    adapter_b [8, 1024, 8]    f32
    adapter_a [8, 8, 1024]    f32
    W0        [1024, 1024]    f32
returns the FULL [8, 2048, 1024] f32 output of:
    BxW0 = einsum('bsd,bdr,do->bsro', x.astype(fp16), adapter_b, W0)
    out  = relu(mean(swapaxes(adapter_a,1,2)[:,None]*BxW0.reshape(b,s,d,r), -1))

Math refactor (verified exactly): with o = kk*128 + g*16 + mp,
    W_eff[dd, o] = adapter_b[dd, kk] * sum_rp adapter_a[rp, o] * W0[dd, (o%128)*8 + rp]
    out[b] = relu((x_fp16[b] @ W_eff[b]) / 8)
which is one [2048,1024] @ [1024,1024] matmul per batch -- data-parallel over
the batch dim: batch b runs on NeuronCore b (sharding_hint's layout).

Host does ONLY data placement (sharding/permutation/replication, no
arithmetic): X^T and W0^T tile-packing, a block-diagonal embedding of
adapter_a (A_sp), and 16x replication of adapter_b (B_bc).

v3 schedule (from the v2 trace: first matmul at 12.1us, ~7.5us of PE gaps
in the build phase, 6.2us store-drain tail; aggregate DMA ~370 GB/s, one
HWDGE queue can't beat ~150 GB/s on 2KB rows -- per-row descriptor cost --
and each DIRECT2D trigger costs ~650ns on its sequencer):
  - Inputs ride THREE queues in parallel: sync (SP HWDGE) carries the W0
    chain then x6..x15; scalar (Act HWDGE) carries asp halves then x0..x5;
    SWDGE (gpsimd) carries bbc. All triggers are emitted up front, so each
    queue FIFO is its own priority list and the W0 chain is never blocked
    behind x-tile data.
  - Cast work is split by engine: DVE does W0-chunk casts + the BTT
    (W_eff = b * C) multiplies, in FIFO order [btt(t-1), cast(t)] (the
    build phase is DMA-bound, so a cast-ahead would head-of-line-block
    the btt behind the next chunk's DMA); the Act engine casts asp and
    x0/x1; x2..x15 casts ride DVE, paired after btts whose data has
    landed, so they never block the chain.
  - 3 warm s-tiles (PSUM: 6 banks + 2 for the C chain) consume W_eff
    chunks as they are built; remaining 13 tiles run back-to-back after,
    4 in flight via the freed C banks.
  - Stores: even tiles -> Act HWDGE, odd tiles -> SWDGE (last SWDGE use
    at tile 13 so its final DRAIN overlaps compute); tiles 14/15 are
    split into 64-row halves across sync+scalar HWDGE queues in parallel
    so the tail after the last matmul is ~1.5us, not 6us.
"""

from contextlib import ExitStack

import numpy as np

S, D, R = 2048, 1024, 8
NT = D // 128
NS = S // 128
WARM = [0, 1]
N_CORES = 8

_compiled = None


def _build_kernel():
    import concourse.bass as bass
    import concourse.tile as tile
    from concourse import bacc, mybir

    F32 = mybir.dt.float32
    F16 = mybir.dt.float16

    nc = bacc.Bacc(
        "TRN2", target_bir_lowering=False, debug=False, num_devices=N_CORES
    )

    x_d = nc.dram_tensor("xtp", [NS, 128, D], F32, kind="ExternalInput").ap()
    w0_d = nc.dram_tensor("w0tp", [NT, 128, D], F32, kind="ExternalInput").ap()
    asp_d = nc.dram_tensor("asp", [128, NT * 128], F32, kind="ExternalInput").ap()
    bbc_d = nc.dram_tensor("bbc", [128, NT * 128], F32, kind="ExternalInput").ap()
    out_d = nc.dram_tensor("out", [S, D], F32, kind="ExternalOutput").ap()

    with tile.TileContext(nc) as tc, ExitStack() as ctx:
        pool = lambda name, bufs, **kw: ctx.enter_context(
            tc.tile_pool(name=name, bufs=bufs, **kw)
        )
        const_p = pool("const", 1)
        w0stage_p = pool("w0stage", 8)
        w0t_p = pool("w0t", 1)
        weff_p = pool("weff", 1)
        xstage_p = pool("xstage", 12)
        xth_p = pool("xth", 6)
        outst_p = pool("outst", 5)
        pmm_w = pool("pmmw", 6, space="PSUM")
        pmm_c = pool("pmmc", 2, space="PSUM")

        import concourse.mybir as mybir_mod

        Copy = mybir_mod.ActivationFunctionType.Copy
        Relu = mybir_mod.ActivationFunctionType.Relu

        # ---------- input DMA triggers, all up front ----------
        # ONE input queue (sync): a single HWDGE queue saturates the
        # ~370 GB/s core HBM share, and parallel queues only SPLIT it
        # (v3a measured the W0 chain at half rate when x rode a second
        # queue). The FIFO order is the priority list. Only bbc rides
        # SWDGE: it is small, off the critical path until btt(0), and
        # SWDGE is otherwise idle until the odd-tile stores.
        bbc = const_p.tile([128, NT * 128], F32, tag="bbc")
        nc.gpsimd.dma_start(bbc[:, 0:128], bbc_d[:, 0:128])

        asp_st = const_p.tile([128, NT * 128], F32, tag="asp_st")
        w0s_tiles = []
        xs_tiles = {}

        def w0_dma(t):
            w0s = w0stage_p.tile([128, D], F32, tag="w0s", name=f"w0s{t}")
            nc.sync.dma_start(w0s[:], w0_d[t])
            w0s_tiles.append(w0s)

        def x_dma(s):
            xs = xstage_p.tile([128, D], F32, tag="xs", name=f"xs{s}")
            nc.sync.dma_start(xs[:], x_d[s])
            xs_tiles[s] = xs

        w0s0 = w0stage_p.tile([128, D], F32, tag="w0s", name="w0s0")
        nc.sync.dma_start(w0s0[:, 0:512], w0_d[0][:, 0:512])
        nc.sync.dma_start(asp_st[:, 0:512], asp_d[:, 0:512])
        nc.sync.dma_start(w0s0[:, 512:1024], w0_d[0][:, 512:1024])
        nc.sync.dma_start(asp_st[:, 512:1024], asp_d[:, 512:1024])
        w0s_tiles.append(w0s0)
        x_dma(0)
        w0_dma(1)
        # bbc bulk rides sync inline at its priority slot: on SWDGE at the
        # head it steals descriptor bandwidth from the critical w0/asp
        # stream (v3b: asp cast slid 2us late)
        nc.sync.dma_start(bbc[:, 128:512], bbc_d[:, 128:512])
        x_dma(1)
        w0_dma(2)
        nc.sync.dma_start(bbc[:, 512:1024], bbc_d[:, 512:1024])
        w0_dma(3)
        w0_dma(4)
        w0_dma(5)
        w0_dma(6)
        w0_dma(7)
        # x3 feeds MAIN tile 3 (first consumer ~W_eff-complete time), and
        # every PSUM bank is held until btt(7): w0_7 gates the main phase,
        # so all non-warm x tiles go after the W0 chain
        for s in range(2, NS):
            x_dma(s)

        # ---------- SBUF fp16 targets ----------
        w0t = w0t_p.tile([128, NT * 1024], F16, tag="w0t")
        weff = weff_p.tile([128, NT * 1024], F16, tag="weff")
        asp_h = const_p.tile([128, NT * 128], F16, tag="asp_h")

        xth_tiles = {}

        def x_cast(s, eng="vector"):
            xth = xth_p.tile([128, D], F16, tag="xth", name=f"xth{s}")
            if eng == "vector":
                nc.vector.tensor_copy(xth[:], xs_tiles[s][:])
            else:
                nc.scalar.activation(xth[:], xs_tiles[s][:], Copy)
            xth_tiles[s] = xth

        def w0_cast(t):
            nc.vector.tensor_copy(
                w0t[:, t * 1024 : (t + 1) * 1024], w0s_tiles[t][:]
            )

        # ---------- per-chunk builders ----------
        pcs_tiles = {}

        def c_alloc(t):
            pcs_tiles[t] = [
                pmm_c.tile([128, 512], F32, tag="pmmc", name=f"pc{t}_{i}")
                for i in range(2)
            ]

        def c_mms(t, gs=range(NT)):
            if t not in pcs_tiles:
                c_alloc(t)
            pcs = pcs_tiles[t]
            for g in gs:
                nc.tensor.matmul(
                    pcs[g // 4][:, (g % 4) * 128 : (g % 4 + 1) * 128],
                    lhsT=w0t[:, t * 1024 + g * 128 : t * 1024 + (g + 1) * 128],
                    rhs=asp_h[:, g * 128 : (g + 1) * 128],
                    start=True,
                    stop=True,
                )

        def btt(t):
            pcs = pcs_tiles[t]
            wv = weff[:, t * 1024 : (t + 1) * 1024].rearrange(
                "p (kk g mp) -> p kk g mp", kk=8, g=NT, mp=16
            )
            for half in range(2):
                wvh = wv[:, :, half * 4 : (half + 1) * 4, :]
                bv = bbc[:, t * 128 : (t + 1) * 128].rearrange(
                    "p (kk mp) -> p kk mp", kk=8
                )[:, :, None, :].broadcast_to([128, 8, 4, 16])
                pv = pcs[half].rearrange("p (g kk mp) -> p kk g mp", g=4, kk=8)
                nc.vector.tensor_tensor(
                    out=wvh, in0=pv, in1=bv, op=mybir_mod.AluOpType.mult
                )

        def mm_pair(po, xth, c):
            for h in range(2):
                nc.tensor.matmul(
                    po[h][:],
                    lhsT=xth[:, c * 128 : (c + 1) * 128],
                    rhs=weff[:, c * 1024 + h * 512 : c * 1024 + (h + 1) * 512],
                    start=(c == 0),
                    stop=(c == NT - 1),
                )

        def evac(s, po):
            outst = outst_p.tile([128, D], F32, tag="outst", name=f"outst{s}")
            nc.scalar.activation(outst[:, 0:512], po[0][:], Relu, scale=0.125)
            nc.vector.tensor_scalar(
                out=outst[:, 512:1024],
                in0=po[1][:],
                scalar1=0.125,
                scalar2=0.0,
                op0=mybir_mod.AluOpType.mult,
                op1=mybir_mod.AluOpType.max,
            )
            return outst

        def store_full(s, outst, eng):
            eng.dma_start(out_d[s * 128 : (s + 1) * 128, :], outst[:])

        def store_split(s, outst, three_way=False):
            # row-split across queues in parallel: rows are 4KB-contiguous
            # DRAM, and per-queue store rate is ~131 GB/s, so splitting is
            # a direct tail win
            if three_way:
                nc.sync.dma_start(
                    out_d[s * 128 : s * 128 + 43, :], outst[0:43, :]
                )
                nc.scalar.dma_start(
                    out_d[s * 128 + 43 : s * 128 + 86, :], outst[43:86, :]
                )
                nc.gpsimd.dma_start(
                    out_d[s * 128 + 86 : (s + 1) * 128, :], outst[86:128, :]
                )
            else:
                nc.sync.dma_start(
                    out_d[s * 128 : s * 128 + 64, :], outst[0:64, :]
                )
                nc.scalar.dma_start(
                    out_d[s * 128 + 64 : (s + 1) * 128, :], outst[64:128, :]
                )

        # ---------- warm phase: W_eff build + 3 warm s-tiles ----------
        po_warm = {
            s: [
                pmm_w.tile([128, 512], F32, tag="pmmw", name=f"po{s}_{i}")
                for i in range(2)
            ]
            for s in WARM
        }

        # chunk 0 at half granularity: each half's casts + C matmuls gate
        # only on that half's DMA
        nc.vector.tensor_copy(w0t[:, 0:512], w0s_tiles[0][:, 0:512])
        nc.scalar.activation(asp_h[:, 0:512], asp_st[:, 0:512], Copy)
        c_mms(0, range(4))
        nc.vector.tensor_copy(w0t[:, 512:1024], w0s_tiles[0][:, 512:1024])
        nc.scalar.activation(asp_h[:, 512:1024], asp_st[:, 512:1024], Copy)
        c_mms(0, range(4, NT))
        btt(0)
        # x0/x1 casts on Act: they are the first pair enablers and the
        # DVE FIFO is busy with the chunk chain
        x_cast(0, eng="scalar")
        x_cast(1, eng="scalar")

        # DVE x-cast pairing: after btt(t), cast the x tile that has
        # surely landed by then, so a cast can never head-of-line-block
        # the next btt in the DVE FIFO
        xc_after_btt = {}

        cursor = {s: 0 for s in WARM}
        joined = {0: 1, 1: 1}  # iteration at which tile s starts pairs

        for t in range(1, NT):
            w0_cast(t)
            c_mms(t)
            btt(t)
            if t in xc_after_btt:
                x_cast(xc_after_btt[t])
            # warm pairs: consume built chunks, up to 2 per tile per
            # iteration, at most 4 per iteration so a stalled pair can
            # never starve the next C(t) for long
            emitted = 0
            for s in WARM:
                if t < joined[s]:
                    continue
                tile_emitted = 0
                while cursor[s] < t and emitted < 4 and tile_emitted < 2:
                    mm_pair(po_warm[s], xth_tiles[s], cursor[s])
                    cursor[s] += 1
                    emitted += 1
                    tile_emitted += 1
        for s in WARM:
            while cursor[s] < NT:
                mm_pair(po_warm[s], xth_tiles[s], cursor[s])
                cursor[s] += 1

        warm_outst = {s: evac(s, po_warm[s]) for s in WARM}
        store_full(0, warm_outst[0], nc.scalar)
        store_full(1, warm_outst[1], nc.gpsimd)

        # ---------- main phase: remaining 13 s-tiles ----------
        for s in range(len(WARM), NS):
            for tgt in (s, s + 1, s + 2):
                if tgt < NS and tgt not in xth_tiles:
                    x_cast(tgt)
            mpool = pmm_c if (s - len(WARM)) % 4 == 3 else pmm_w
            tg = "pmmc" if mpool is pmm_c else "pmmw"
            po = [
                mpool.tile([128, 512], F32, tag=tg, name=f"po{s}_{i}")
                for i in range(2)
            ]
            for c in range(NT):
                mm_pair(po, xth_tiles[s], c)
            outst = evac(s, po)
            if s >= NS - 2:
                # HWDGE queues only: a SWDGE store this late adds a ~7us
                # final DRAIN on the GpSimd sequencer (measured v3c)
                store_split(s, outst)
            elif s % 2 == 1:
                store_full(s, outst, nc.gpsimd)
            else:
                store_full(s, outst, nc.scalar)

    nc.compile()
    return nc


def _pack_inputs(x_b, adapter_b_b, adapter_a_b, W0):
    """Pure data placement (permutation / replication / zero-padding)."""
    xtp = np.ascontiguousarray(
        x_b.reshape(NS, 128, NT, 128).transpose(0, 3, 2, 1).reshape(NS, 128, D),
        np.float32,
    )
    w0tp = np.ascontiguousarray(
        W0.reshape(NT, 128, NT, 128).transpose(0, 3, 2, 1).reshape(NT, 128, D),
        np.float32,
    )
    asp = np.zeros((NT, 128, 128), np.float32)
    aa = adapter_a_b
    for g in range(NT):
        for mp in range(16):
            for rp in range(R):
                asp[g, mp * 8 + rp, np.arange(8) * 16 + mp] = aa[
                    rp, np.arange(8) * 128 + g * 16 + mp
                ]
    asp = np.ascontiguousarray(asp.transpose(1, 0, 2).reshape(128, NT * 128))
    bbc = np.repeat(adapter_b_b, 16, axis=1).reshape(D, 128)
    bbc = np.ascontiguousarray(
        bbc.reshape(NT, 128, 128).transpose(1, 0, 2).reshape(128, NT * 128),
        np.float32,
    )
    return {"xtp": xtp, "w0tp": w0tp, "asp": asp, "bbc": bbc}


def kernel(x, adapter_b, adapter_a, W0):
    global _compiled
    x = np.asarray(x, np.float32)
    adapter_b = np.asarray(adapter_b, np.float32)
    adapter_a = np.asarray(adapter_a, np.float32)
    W0 = np.asarray(W0, np.float32)
    B = x.shape[0]
    assert B == N_CORES and x.shape == (B, S, D)

    if _compiled is None:
        _compiled = _build_kernel()

    from concourse.bass_utils import run_bass_kernel_spmd

    in_maps = [
        _pack_inputs(x[b], adapter_b[b], adapter_a[b], W0) for b in range(B)
    ]
    res = run_bass_kernel_spmd(_compiled, in_maps, list(range(N_CORES)))
    out = np.stack([res.results[b]["out"] for b in range(B)]).astype(np.float32)
    return out


# revision 19
# speedup vs baseline: 1.0343x; 1.0343x over previous
"""Self-contained TRN2 Bass kernel for nn_FLoRALayer (B=8, S=2048, D=1024, R=8).

kernel(**inputs) takes FULL unsharded inputs:
    x         [8, 2048, 1024] f32
    adapter_b [8, 1024, 8]    f32
    adapter_a [8, 8, 1024]    f32
    W0        [1024, 1024]    f32
returns the FULL [8, 2048, 1024] f32 output of:
    BxW0 = einsum('bsd,bdr,do->bsro', x.astype(fp16), adapter_b, W0)
    out  = relu(mean(swapaxes(adapter_a,1,2)[:,None]*BxW0.reshape(b,s,d,r), -1))

Math refactor (verified exactly): with o = kk*128 + g*16 + mp,
    W_eff[dd, o] = adapter_b[dd, kk] * sum_rp adapter_a[rp, o] * W0[dd, (o%128)*8 + rp]
    out[b] = relu((x_fp16[b] @ W_eff[b]) / 8)
which is one [2048,1024] @ [1024,1024] matmul per batch -- data-parallel over
the batch dim: batch b runs on NeuronCore b (sharding_hint's layout).

Host does ONLY data placement (sharding/permutation/replication, no
arithmetic): X^T and W0^T tile-packing, a block-diagonal embedding of
adapter_a (A_sp), and 16x replication of adapter_b (B_bc).

v3 schedule (from the v2 trace: first matmul at 12.1us, ~7.5us of PE gaps
in the build phase, 6.2us store-drain tail; aggregate DMA ~370 GB/s, one
HWDGE queue can't beat ~150 GB/s on 2KB rows -- per-row descriptor cost --
and each DIRECT2D trigger costs ~650ns on its sequencer):
  - Inputs ride THREE queues in parallel: sync (SP HWDGE) carries the W0
    chain then x6..x15; scalar (Act HWDGE) carries asp halves then x0..x5;
    SWDGE (gpsimd) carries bbc. All triggers are emitted up front, so each
    queue FIFO is its own priority list and the W0 chain is never blocked
    behind x-tile data.
  - Cast work is split by engine: DVE does W0-chunk casts + the BTT
    (W_eff = b * C) multiplies, in FIFO order [btt(t-1), cast(t)] (the
    build phase is DMA-bound, so a cast-ahead would head-of-line-block
    the btt behind the next chunk's DMA); the Act engine casts asp and
    x0/x1; x2..x15 casts ride DVE, paired after btts whose data has
    landed, so they never block the chain.
  - 3 warm s-tiles (PSUM: 6 banks + 2 for the C chain) consume W_eff
    chunks as they are built; remaining 13 tiles run back-to-back after,
    4 in flight via the freed C banks.
  - Stores: even tiles -> Act HWDGE, odd tiles -> SWDGE (last SWDGE use
    at tile 13 so its final DRAIN overlaps compute); tiles 14/15 are
    split into 64-row halves across sync+scalar HWDGE queues in parallel
    so the tail after the last matmul is ~1.5us, not 6us.
"""

from contextlib import ExitStack

import numpy as np

S, D, R = 2048, 1024, 8
NT = D // 128
NS = S // 128
WARM = [0, 1, 2]
N_CORES = 8

_compiled = None


def _build_kernel():
    import concourse.bass as bass
    import concourse.tile as tile
    from concourse import bacc, mybir

    F32 = mybir.dt.float32
    F16 = mybir.dt.float16

    nc = bacc.Bacc(
        "TRN2", target_bir_lowering=False, debug=False, num_devices=N_CORES
    )

    x_d = nc.dram_tensor("xtp", [NS, 128, D], F32, kind="ExternalInput").ap()
    w0_d = nc.dram_tensor("w0tp", [NT, 128, D], F32, kind="ExternalInput").ap()
    asp_d = nc.dram_tensor("asp", [128, NT * 128], F32, kind="ExternalInput").ap()
    bbc_d = nc.dram_tensor("bbc", [128, NT * 128], F32, kind="ExternalInput").ap()
    out_d = nc.dram_tensor("out", [S, D], F32, kind="ExternalOutput").ap()

    with tile.TileContext(nc) as tc, ExitStack() as ctx:
        pool = lambda name, bufs, **kw: ctx.enter_context(
            tc.tile_pool(name=name, bufs=bufs, **kw)
        )
        const_p = pool("const", 1)
        w0stage_p = pool("w0stage", 8)
        w0t_p = pool("w0t", 1)
        weff_p = pool("weff", 1)
        xstage_p = pool("xstage", 12)
        xth_p = pool("xth", 6)
        outst_p = pool("outst", 5)
        pmm_w = pool("pmmw", 6, space="PSUM")
        pmm_c = pool("pmmc", 2, space="PSUM")

        import concourse.mybir as mybir_mod

        Copy = mybir_mod.ActivationFunctionType.Copy
        Relu = mybir_mod.ActivationFunctionType.Relu

        # ---------- input DMA triggers, all up front ----------
        # ONE input queue (sync): a single HWDGE queue saturates the
        # ~370 GB/s core HBM share, and parallel queues only SPLIT it
        # (v3a measured the W0 chain at half rate when x rode a second
        # queue). The FIFO order is the priority list. Only bbc rides
        # SWDGE: it is small, off the critical path until btt(0), and
        # SWDGE is otherwise idle until the odd-tile stores.
        bbc = const_p.tile([128, NT * 128], F32, tag="bbc")
        nc.gpsimd.dma_start(bbc[:, 0:128], bbc_d[:, 0:128])

        asp_st = const_p.tile([128, NT * 128], F32, tag="asp_st")
        w0s_tiles = []
        xs_tiles = {}

        def w0_dma(t):
            w0s = w0stage_p.tile([128, D], F32, tag="w0s", name=f"w0s{t}")
            nc.sync.dma_start(w0s[:], w0_d[t])
            w0s_tiles.append(w0s)

        def x_dma(s):
            xs = xstage_p.tile([128, D], F32, tag="xs", name=f"xs{s}")
            nc.sync.dma_start(xs[:], x_d[s])
            xs_tiles[s] = xs

        w0s0 = w0stage_p.tile([128, D], F32, tag="w0s", name="w0s0")
        nc.sync.dma_start(w0s0[:, 0:512], w0_d[0][:, 0:512])
        nc.sync.dma_start(asp_st[:, 0:512], asp_d[:, 0:512])
        nc.sync.dma_start(asp_st[:, 512:1024], asp_d[:, 512:1024])
        nc.sync.dma_start(w0s0[:, 512:1024], w0_d[0][:, 512:1024])
        w0s_tiles.append(w0s0)
        x_dma(0)
        w0_dma(1)
        nc.sync.dma_start(bbc[:, 128:512], bbc_d[:, 128:512])
        x_dma(1)
        w0_dma(2)
        nc.sync.dma_start(bbc[:, 512:1024], bbc_d[:, 512:1024])
        x_dma(2)
        w0_dma(3)
        w0_dma(4)
        w0_dma(5)
        w0_dma(6)
        w0_dma(7)
        # x3.. feed MAIN tiles (first consumer ~W_eff-complete time), and
        # every PSUM bank is held until btt(7): w0_7 gates the main phase,
        # so all non-warm x tiles go after the W0 chain
        for s in range(3, NS):
            x_dma(s)

        # ---------- SBUF fp16 targets ----------
        w0t = w0t_p.tile([128, NT * 1024], F16, tag="w0t")
        weff = weff_p.tile([128, NT * 1024], F16, tag="weff")
        asp_h = const_p.tile([128, NT * 128], F16, tag="asp_h")

        xth_tiles = {}

        def x_cast(s, eng="vector"):
            xth = xth_p.tile([128, D], F16, tag="xth", name=f"xth{s}")
            if eng == "vector":
                nc.vector.tensor_copy(xth[:], xs_tiles[s][:])
            else:
                nc.scalar.activation(xth[:], xs_tiles[s][:], Copy)
            xth_tiles[s] = xth

        def w0_cast(t):
            nc.vector.tensor_copy(
                w0t[:, t * 1024 : (t + 1) * 1024], w0s_tiles[t][:]
            )

        # ---------- per-chunk builders ----------
        pcs_tiles = {}

        def c_alloc(t):
            pcs_tiles[t] = [
                pmm_c.tile([128, 512], F32, tag="pmmc", name=f"pc{t}_{i}")
                for i in range(2)
            ]

        def c_mms(t, gs=range(NT)):
            if t not in pcs_tiles:
                c_alloc(t)
            pcs = pcs_tiles[t]
            for g in gs:
                nc.tensor.matmul(
                    pcs[g // 4][:, (g % 4) * 128 : (g % 4 + 1) * 128],
                    lhsT=w0t[:, t * 1024 + g * 128 : t * 1024 + (g + 1) * 128],
                    rhs=asp_h[:, g * 128 : (g + 1) * 128],
                    start=True,
                    stop=True,
                )

        def btt(t):
            pcs = pcs_tiles[t]
            wv = weff[:, t * 1024 : (t + 1) * 1024].rearrange(
                "p (kk g mp) -> p kk g mp", kk=8, g=NT, mp=16
            )
            for half in range(2):
                wvh = wv[:, :, half * 4 : (half + 1) * 4, :]
                bv = bbc[:, t * 128 : (t + 1) * 128].rearrange(
                    "p (kk mp) -> p kk mp", kk=8
                )[:, :, None, :].broadcast_to([128, 8, 4, 16])
                pv = pcs[half].rearrange("p (g kk mp) -> p kk g mp", g=4, kk=8)
                nc.vector.tensor_tensor(
                    out=wvh, in0=pv, in1=bv, op=mybir_mod.AluOpType.mult
                )

        def mm_pair(po, xth, c):
            for h in range(2):
                nc.tensor.matmul(
                    po[h][:],
                    lhsT=xth[:, c * 128 : (c + 1) * 128],
                    rhs=weff[:, c * 1024 + h * 512 : c * 1024 + (h + 1) * 512],
                    start=(c == 0),
                    stop=(c == NT - 1),
                )

        def evac(s, po):
            outst = outst_p.tile([128, D], F32, tag="outst", name=f"outst{s}")
            nc.scalar.activation(outst[:, 0:512], po[0][:], Relu, scale=0.125)
            nc.vector.tensor_scalar(
                out=outst[:, 512:1024],
                in0=po[1][:],
                scalar1=0.125,
                scalar2=0.0,
                op0=mybir_mod.AluOpType.mult,
                op1=mybir_mod.AluOpType.max,
            )
            return outst

        def store_full(s, outst, eng):
            eng.dma_start(out_d[s * 128 : (s + 1) * 128, :], outst[:])

        def store_split(s, outst, three_way=False):
            # row-split across queues in parallel: rows are 4KB-contiguous
            # DRAM, and per-queue store rate is ~131 GB/s, so splitting is
            # a direct tail win
            if three_way:
                nc.sync.dma_start(
                    out_d[s * 128 : s * 128 + 43, :], outst[0:43, :]
                )
                nc.scalar.dma_start(
                    out_d[s * 128 + 43 : s * 128 + 86, :], outst[43:86, :]
                )
                nc.gpsimd.dma_start(
                    out_d[s * 128 + 86 : (s + 1) * 128, :], outst[86:128, :]
                )
            else:
                for i, eng in enumerate((nc.sync, nc.scalar, nc.sync, nc.scalar)):
                    r0, r1 = i * 32, (i + 1) * 32
                    eng.dma_start(
                        out_d[s * 128 + r0 : s * 128 + r1, :], outst[r0:r1, :]
                    )

        # ---------- warm phase: W_eff build + 3 warm s-tiles ----------
        po_warm = {
            s: [
                pmm_w.tile([128, 512], F32, tag="pmmw", name=f"po{s}_{i}")
                for i in range(2)
            ]
            for s in WARM
        }

        # chunk 0 at half granularity: each half's casts + C matmuls gate
        # only on that half's DMA
        nc.vector.tensor_copy(w0t[:, 0:512], w0s_tiles[0][:, 0:512])
        nc.scalar.activation(asp_h[:, 0:512], asp_st[:, 0:512], Copy)
        c_mms(0, range(4))
        nc.vector.tensor_copy(w0t[:, 512:1024], w0s_tiles[0][:, 512:1024])
        nc.scalar.activation(asp_h[:, 512:1024], asp_st[:, 512:1024], Copy)
        c_mms(0, range(4, NT))
        btt(0)
        # x0/x1 casts on Act: they are the first pair enablers and the
        # DVE FIFO is busy with the chunk chain
        x_cast(0, eng="scalar")
        x_cast(1, eng="scalar")

        # DVE x-cast pairing: after btt(t), cast the x tile that has
        # surely landed by then, so a cast can never head-of-line-block
        # the next btt in the DVE FIFO
        xc_after_btt = {3: 2, 5: 3}

        cursor = {s: 0 for s in WARM}
        joined = {0: 1, 1: 1, 2: 4}  # iteration at which tile s starts pairs

        for t in range(1, NT):
            w0_cast(t)
            c_mms(t)
            btt(t)
            if t in xc_after_btt:
                x_cast(xc_after_btt[t])
            # warm pairs: consume built chunks, up to 2 per tile per
            # iteration, at most 4 per iteration so a stalled pair can
            # never starve the next C(t) for long
            emitted = 0
            for s in WARM:
                if t < joined[s]:
                    continue
                tile_emitted = 0
                while cursor[s] < t and emitted < 4 and tile_emitted < 2:
                    mm_pair(po_warm[s], xth_tiles[s], cursor[s])
                    cursor[s] += 1
                    emitted += 1
                    tile_emitted += 1
        for s in WARM:
            while cursor[s] < NT:
                mm_pair(po_warm[s], xth_tiles[s], cursor[s])
                cursor[s] += 1

        warm_outst = {s: evac(s, po_warm[s]) for s in WARM}
        store_full(0, warm_outst[0], nc.scalar)
        store_full(1, warm_outst[1], nc.gpsimd)
        store_full(2, warm_outst[2], nc.scalar)

        # ---------- main phase: remaining 13 s-tiles ----------
        for s in range(len(WARM), NS):
            for tgt in (s, s + 1, s + 2):
                if tgt < NS and tgt not in xth_tiles:
                    x_cast(tgt)
            mpool = pmm_c if (s - len(WARM)) % 4 == 3 else pmm_w
            tg = "pmmc" if mpool is pmm_c else "pmmw"
            po = [
                mpool.tile([128, 512], F32, tag=tg, name=f"po{s}_{i}")
                for i in range(2)
            ]
            for c in range(NT):
                mm_pair(po, xth_tiles[s], c)
            outst = evac(s, po)
            if s >= NS - 2:
                # HWDGE queues only: a SWDGE store this late adds a ~7us
                # final DRAIN on the GpSimd sequencer (measured v3c)
                store_split(s, outst)
            elif s % 2 == 1:
                store_full(s, outst, nc.gpsimd)
            else:
                store_full(s, outst, nc.scalar)

    nc.compile()
    return nc


def _pack_inputs(x_b, adapter_b_b, adapter_a_b, W0):
    """Pure data placement (permutation / replication / zero-padding)."""
    xtp = np.ascontiguousarray(
        x_b.reshape(NS, 128, NT, 128).transpose(0, 3, 2, 1).reshape(NS, 128, D),
        np.float32,
    )
    w0tp = np.ascontiguousarray(
        W0.reshape(NT, 128, NT, 128).transpose(0, 3, 2, 1).reshape(NT, 128, D),
        np.float32,
    )
    asp = np.zeros((NT, 128, 128), np.float32)
    aa = adapter_a_b
    for g in range(NT):
        for mp in range(16):
            for rp in range(R):
                asp[g, mp * 8 + rp, np.arange(8) * 16 + mp] = aa[
                    rp, np.arange(8) * 128 + g * 16 + mp
                ]
    asp = np.ascontiguousarray(asp.transpose(1, 0, 2).reshape(128, NT * 128))
    bbc = np.repeat(adapter_b_b, 16, axis=1).reshape(D, 128)
    bbc = np.ascontiguousarray(
        bbc.reshape(NT, 128, 128).transpose(1, 0, 2).reshape(128, NT * 128),
        np.float32,
    )
    return {"xtp": xtp, "w0tp": w0tp, "asp": asp, "bbc": bbc}


def kernel(x, adapter_b, adapter_a, W0):
    global _compiled
    x = np.asarray(x, np.float32)
    adapter_b = np.asarray(adapter_b, np.float32)
    adapter_a = np.asarray(adapter_a, np.float32)
    W0 = np.asarray(W0, np.float32)
    B = x.shape[0]
    assert B == N_CORES and x.shape == (B, S, D)

    if _compiled is None:
        _compiled = _build_kernel()

    from concourse.bass_utils import run_bass_kernel_spmd

    in_maps = [
        _pack_inputs(x[b], adapter_b[b], adapter_a[b], W0) for b in range(B)
    ]
    res = run_bass_kernel_spmd(_compiled, in_maps, list(range(N_CORES)))
    out = np.stack([res.results[b]["out"] for b in range(B)]).astype(np.float32)
    return out


# revision 21
# speedup vs baseline: 1.0361x; 1.0018x over previous
"""Self-contained TRN2 Bass kernel for nn_FLoRALayer (B=8, S=2048, D=1024, R=8).

kernel(**inputs) takes FULL unsharded inputs:
    x         [8, 2048, 1024] f32
    adapter_b [8, 1024, 8]    f32
    adapter_a [8, 8, 1024]    f32
    W0        [1024, 1024]    f32
returns the FULL [8, 2048, 1024] f32 output of:
    BxW0 = einsum('bsd,bdr,do->bsro', x.astype(fp16), adapter_b, W0)
    out  = relu(mean(swapaxes(adapter_a,1,2)[:,None]*BxW0.reshape(b,s,d,r), -1))

Math refactor (verified exactly): with o = kk*128 + g*16 + mp,
    W_eff[dd, o] = adapter_b[dd, kk] * sum_rp adapter_a[rp, o] * W0[dd, (o%128)*8 + rp]
    out[b] = relu((x_fp16[b] @ W_eff[b]) / 8)
which is one [2048,1024] @ [1024,1024] matmul per batch -- data-parallel over
the batch dim: batch b runs on NeuronCore b (sharding_hint's layout).

Host does ONLY data placement (sharding/permutation/replication, no
arithmetic): X^T and W0^T tile-packing, a block-diagonal embedding of
adapter_a (A_sp), and 16x replication of adapter_b (B_bc).

v3 schedule (from the v2 trace: first matmul at 12.1us, ~7.5us of PE gaps
in the build phase, 6.2us store-drain tail; aggregate DMA ~370 GB/s, one
HWDGE queue can't beat ~150 GB/s on 2KB rows -- per-row descriptor cost --
and each DIRECT2D trigger costs ~650ns on its sequencer):
  - Inputs ride THREE queues in parallel: sync (SP HWDGE) carries the W0
    chain then x6..x15; scalar (Act HWDGE) carries asp halves then x0..x5;
    SWDGE (gpsimd) carries bbc. All triggers are emitted up front, so each
    queue FIFO is its own priority list and the W0 chain is never blocked
    behind x-tile data.
  - Cast work is split by engine: DVE does W0-chunk casts + the BTT
    (W_eff = b * C) multiplies, in FIFO order [btt(t-1), cast(t)] (the
    build phase is DMA-bound, so a cast-ahead would head-of-line-block
    the btt behind the next chunk's DMA); the Act engine casts asp and
    x0/x1; x2..x15 casts ride DVE, paired after btts whose data has
    landed, so they never block the chain.
  - 3 warm s-tiles (PSUM: 6 banks + 2 for the C chain) consume W_eff
    chunks as they are built; remaining 13 tiles run back-to-back after,
    4 in flight via the freed C banks.
  - Stores: even tiles -> Act HWDGE, odd tiles -> SWDGE (last SWDGE use
    at tile 13 so its final DRAIN overlaps compute); tiles 14/15 are
    split into 64-row halves across sync+scalar HWDGE queues in parallel
    so the tail after the last matmul is ~1.5us, not 6us.
"""

from contextlib import ExitStack

import numpy as np

S, D, R = 2048, 1024, 8
NT = D // 128
NS = S // 128
WARM = [0, 1, 2]
N_CORES = 8

_compiled = None


def _build_kernel():
    import concourse.bass as bass
    import concourse.tile as tile
    from concourse import bacc, mybir

    F32 = mybir.dt.float32
    F16 = mybir.dt.float16

    nc = bacc.Bacc(
        "TRN2", target_bir_lowering=False, debug=False, num_devices=N_CORES
    )

    x_d = nc.dram_tensor("xtp", [NS, 128, D], F32, kind="ExternalInput").ap()
    w0_d = nc.dram_tensor("w0tp", [NT, 128, D], F32, kind="ExternalInput").ap()
    asp_d = nc.dram_tensor("asp", [128, NT * 128], F32, kind="ExternalInput").ap()
    bbc_d = nc.dram_tensor("bbc", [128, NT * R], F32, kind="ExternalInput").ap()
    out_d = nc.dram_tensor("out", [S, D], F32, kind="ExternalOutput").ap()

    with tile.TileContext(nc) as tc, ExitStack() as ctx:
        pool = lambda name, bufs, **kw: ctx.enter_context(
            tc.tile_pool(name=name, bufs=bufs, **kw)
        )
        const_p = pool("const", 1)
        w0stage_p = pool("w0stage", 8)
        w0t_p = pool("w0t", 1)
        weff_p = pool("weff", 1)
        xstage_p = pool("xstage", 12)
        xth_p = pool("xth", 6)
        outst_p = pool("outst", 5)
        pmm_w = pool("pmmw", 6, space="PSUM")
        pmm_c = pool("pmmc", 2, space="PSUM")

        import concourse.mybir as mybir_mod

        Copy = mybir_mod.ActivationFunctionType.Copy
        Relu = mybir_mod.ActivationFunctionType.Relu

        # ---------- input DMA triggers, all up front ----------
        # ONE input queue (sync): a single HWDGE queue saturates the
        # ~370 GB/s core HBM share, and parallel queues only SPLIT it
        # (v3a measured the W0 chain at half rate when x rode a second
        # queue). The FIFO order is the priority list. Only bbc rides
        # SWDGE: it is small, off the critical path until btt(0), and
        # SWDGE is otherwise idle until the odd-tile stores.
        bbc = const_p.tile([128, NT * 128], F32, tag="bbc")
        abr = const_p.tile([128, NT * R], F32, tag="abr")
        nc.gpsimd.dma_start(abr[:], bbc_d[:])

        asp_st = const_p.tile([128, NT * 128], F32, tag="asp_st")
        w0s_tiles = []
        xs_tiles = {}

        def w0_dma(t):
            w0s = w0stage_p.tile([128, D], F32, tag="w0s", name=f"w0s{t}")
            nc.sync.dma_start(w0s[:], w0_d[t])
            w0s_tiles.append(w0s)

        def x_dma(s):
            xs = xstage_p.tile([128, D], F32, tag="xs", name=f"xs{s}")
            nc.sync.dma_start(xs[:], x_d[s])
            xs_tiles[s] = xs

        w0s0 = w0stage_p.tile([128, D], F32, tag="w0s", name="w0s0")
        nc.sync.dma_start(w0s0[:, 0:512], w0_d[0][:, 0:512])
        nc.sync.dma_start(asp_st[:, 0:512], asp_d[:, 0:512])
        nc.sync.dma_start(asp_st[:, 512:1024], asp_d[:, 512:1024])
        nc.sync.dma_start(w0s0[:, 512:1024], w0_d[0][:, 512:1024])
        w0s_tiles.append(w0s0)
        x_dma(0)
        w0_dma(1)
        x_dma(1)
        w0_dma(2)
        x_dma(2)
        w0_dma(3)
        w0_dma(4)
        w0_dma(5)
        w0_dma(6)
        w0_dma(7)
        # x3.. feed MAIN tiles (first consumer ~W_eff-complete time), and
        # every PSUM bank is held until btt(7): w0_7 gates the main phase,
        # so all non-warm x tiles go after the W0 chain
        for s in range(3, NS):
            x_dma(s)

        # ---------- SBUF fp16 targets ----------
        w0t = w0t_p.tile([128, NT * 1024], F16, tag="w0t")
        weff = weff_p.tile([128, NT * 1024], F16, tag="weff")
        asp_h = const_p.tile([128, NT * 128], F16, tag="asp_h")

        xth_tiles = {}

        def x_cast(s, eng="vector"):
            xth = xth_p.tile([128, D], F16, tag="xth", name=f"xth{s}")
            if eng == "vector":
                nc.vector.tensor_copy(xth[:], xs_tiles[s][:])
            else:
                nc.scalar.activation(xth[:], xs_tiles[s][:], Copy)
            xth_tiles[s] = xth

        def w0_cast(t):
            nc.vector.tensor_copy(
                w0t[:, t * 1024 : (t + 1) * 1024], w0s_tiles[t][:]
            )

        # ---------- per-chunk builders ----------
        pcs_tiles = {}

        def c_alloc(t):
            pcs_tiles[t] = [
                pmm_c.tile([128, 512], F32, tag="pmmc", name=f"pc{t}_{i}")
                for i in range(2)
            ]

        def c_mms(t, gs=range(NT)):
            if t not in pcs_tiles:
                c_alloc(t)
            pcs = pcs_tiles[t]
            for g in gs:
                nc.tensor.matmul(
                    pcs[g // 4][:, (g % 4) * 128 : (g % 4 + 1) * 128],
                    lhsT=w0t[:, t * 1024 + g * 128 : t * 1024 + (g + 1) * 128],
                    rhs=asp_h[:, g * 128 : (g + 1) * 128],
                    start=True,
                    stop=True,
                )

        def btt(t):
            pcs = pcs_tiles[t]
            wv = weff[:, t * 1024 : (t + 1) * 1024].rearrange(
                "p (kk g mp) -> p kk g mp", kk=8, g=NT, mp=16
            )
            for half in range(2):
                wvh = wv[:, :, half * 4 : (half + 1) * 4, :]
                bv = bbc[:, t * 128 : (t + 1) * 128].rearrange(
                    "p (kk mp) -> p kk mp", kk=8
                )[:, :, None, :].broadcast_to([128, 8, 4, 16])
                pv = pcs[half].rearrange("p (g kk mp) -> p kk g mp", g=4, kk=8)
                nc.vector.tensor_tensor(
                    out=wvh, in0=pv, in1=bv, op=mybir_mod.AluOpType.mult
                )

        def mm_pair(po, xth, c):
            for h in range(2):
                nc.tensor.matmul(
                    po[h][:],
                    lhsT=xth[:, c * 128 : (c + 1) * 128],
                    rhs=weff[:, c * 1024 + h * 512 : c * 1024 + (h + 1) * 512],
                    start=(c == 0),
                    stop=(c == NT - 1),
                )

        def evac(s, po):
            outst = outst_p.tile([128, D], F32, tag="outst", name=f"outst{s}")
            nc.scalar.activation(outst[:, 0:512], po[0][:], Relu, scale=0.125)
            nc.vector.tensor_scalar(
                out=outst[:, 512:1024],
                in0=po[1][:],
                scalar1=0.125,
                scalar2=0.0,
                op0=mybir_mod.AluOpType.mult,
                op1=mybir_mod.AluOpType.max,
            )
            return outst

        def store_full(s, outst, eng):
            eng.dma_start(out_d[s * 128 : (s + 1) * 128, :], outst[:])

        def store_split(s, outst, three_way=False):
            # row-split across queues in parallel: rows are 4KB-contiguous
            # DRAM, and per-queue store rate is ~131 GB/s, so splitting is
            # a direct tail win
            if three_way:
                nc.sync.dma_start(
                    out_d[s * 128 : s * 128 + 43, :], outst[0:43, :]
                )
                nc.scalar.dma_start(
                    out_d[s * 128 + 43 : s * 128 + 86, :], outst[43:86, :]
                )
                nc.gpsimd.dma_start(
                    out_d[s * 128 + 86 : (s + 1) * 128, :], outst[86:128, :]
                )
            else:
                for i, eng in enumerate((nc.sync, nc.scalar, nc.sync, nc.scalar)):
                    r0, r1 = i * 32, (i + 1) * 32
                    eng.dma_start(
                        out_d[s * 128 + r0 : s * 128 + r1, :], outst[r0:r1, :]
                    )

        # ---------- warm phase: W_eff build + 3 warm s-tiles ----------
        po_warm = {
            s: [
                pmm_w.tile([128, 512], F32, tag="pmmw", name=f"po{s}_{i}")
                for i in range(2)
            ]
            for s in WARM
        }

        # bbc = 16x on-device broadcast of raw adapter_b (saves 480KB of
        # critical head-stream DMA; host used to ship the replicated form)
        nc.vector.tensor_copy(
            bbc.rearrange("p (t kk mp) -> p t kk mp", t=NT, kk=R, mp=16),
            abr.rearrange("p (t kk) -> p t kk", t=NT)[:, :, :, None]
            .broadcast_to([128, NT, R, 16]),
        )

        # chunk 0 at half granularity: each half's casts + C matmuls gate
        # only on that half's DMA
        nc.vector.tensor_copy(w0t[:, 0:512], w0s_tiles[0][:, 0:512])
        nc.scalar.activation(asp_h[:, 0:512], asp_st[:, 0:512], Copy)
        c_mms(0, range(4))
        nc.vector.tensor_copy(w0t[:, 512:1024], w0s_tiles[0][:, 512:1024])
        nc.scalar.activation(asp_h[:, 512:1024], asp_st[:, 512:1024], Copy)
        c_mms(0, range(4, NT))
        btt(0)
        # x0/x1 casts on Act: they are the first pair enablers and the
        # DVE FIFO is busy with the chunk chain
        x_cast(0, eng="scalar")
        x_cast(1, eng="scalar")

        # DVE x-cast pairing: after btt(t), cast the x tile that has
        # surely landed by then, so a cast can never head-of-line-block
        # the next btt in the DVE FIFO
        xc_after_btt = {3: 2, 5: 3}

        cursor = {s: 0 for s in WARM}
        joined = {0: 1, 1: 1, 2: 4}  # iteration at which tile s starts pairs

        for t in range(1, NT):
            w0_cast(t)
            c_mms(t)
            btt(t)
            if t in xc_after_btt:
                x_cast(xc_after_btt[t])
            # warm pairs: consume built chunks, up to 2 per tile per
            # iteration, at most 4 per iteration so a stalled pair can
            # never starve the next C(t) for long
            emitted = 0
            for s in WARM:
                if t < joined[s]:
                    continue
                tile_emitted = 0
                while cursor[s] < t and emitted < 4 and tile_emitted < 2:
                    mm_pair(po_warm[s], xth_tiles[s], cursor[s])
                    cursor[s] += 1
                    emitted += 1
                    tile_emitted += 1
        for s in WARM:
            while cursor[s] < NT:
                mm_pair(po_warm[s], xth_tiles[s], cursor[s])
                cursor[s] += 1

        warm_outst = {s: evac(s, po_warm[s]) for s in WARM}
        store_full(0, warm_outst[0], nc.scalar)
        store_full(1, warm_outst[1], nc.gpsimd)
        store_full(2, warm_outst[2], nc.scalar)

        # ---------- main phase: remaining 13 s-tiles ----------
        for s in range(len(WARM), NS):
            for tgt in (s, s + 1, s + 2):
                if tgt < NS and tgt not in xth_tiles:
                    x_cast(tgt)
            mpool = pmm_c if (s - len(WARM)) % 4 == 3 else pmm_w
            tg = "pmmc" if mpool is pmm_c else "pmmw"
            po = [
                mpool.tile([128, 512], F32, tag=tg, name=f"po{s}_{i}")
                for i in range(2)
            ]
            for c in range(NT):
                mm_pair(po, xth_tiles[s], c)
            outst = evac(s, po)
            if s >= NS - 2:
                # HWDGE queues only: a SWDGE store this late adds a ~7us
                # final DRAIN on the GpSimd sequencer (measured v3c)
                store_split(s, outst)
            elif s % 2 == 1:
                store_full(s, outst, nc.gpsimd)
            else:
                store_full(s, outst, nc.scalar)

    nc.compile()
    return nc


def _pack_inputs(x_b, adapter_b_b, adapter_a_b, W0):
    """Pure data placement (permutation / replication / zero-padding)."""
    xtp = np.ascontiguousarray(
        x_b.reshape(NS, 128, NT, 128).transpose(0, 3, 2, 1).reshape(NS, 128, D),
        np.float32,
    )
    w0tp = np.ascontiguousarray(
        W0.reshape(NT, 128, NT, 128).transpose(0, 3, 2, 1).reshape(NT, 128, D),
        np.float32,
    )
    asp = np.zeros((NT, 128, 128), np.float32)
    aa = adapter_a_b
    for g in range(NT):
        for mp in range(16):
            for rp in range(R):
                asp[g, mp * 8 + rp, np.arange(8) * 16 + mp] = aa[
                    rp, np.arange(8) * 128 + g * 16 + mp
                ]
    asp = np.ascontiguousarray(asp.transpose(1, 0, 2).reshape(128, NT * 128))
    # raw adapter_b, permuted to [dd, (t, kk)]; the kernel broadcasts the
    # 16x mp replication on-device
    bbc = np.ascontiguousarray(
        adapter_b_b.reshape(NT, 128, R).transpose(1, 0, 2).reshape(128, NT * R),
        np.float32,
    )
    return {"xtp": xtp, "w0tp": w0tp, "asp": asp, "bbc": bbc}


def kernel(x, adapter_b, adapter_a, W0):
    global _compiled
    x = np.asarray(x, np.float32)
    adapter_b = np.asarray(adapter_b, np.float32)
    adapter_a = np.asarray(adapter_a, np.float32)
    W0 = np.asarray(W0, np.float32)
    B = x.shape[0]
    assert B == N_CORES and x.shape == (B, S, D)

    if _compiled is None:
        _compiled = _build_kernel()

    from concourse.bass_utils import run_bass_kernel_spmd

    in_maps = [
        _pack_inputs(x[b], adapter_b[b], adapter_a[b], W0) for b in range(B)
    ]
    res = run_bass_kernel_spmd(_compiled, in_maps, list(range(N_CORES)))
    out = np.stack([res.results[b]["out"] for b in range(B)]).astype(np.float32)
    return out


# revision 22
# speedup vs baseline: 1.0480x; 1.0115x over previous
"""Self-contained TRN2 Bass kernel for nn_FLoRALayer (B=8, S=2048, D=1024, R=8).

kernel(**inputs) takes FULL unsharded inputs:
    x         [8, 2048, 1024] f32
    adapter_b [8, 1024, 8]    f32
    adapter_a [8, 8, 1024]    f32
    W0        [1024, 1024]    f32
returns the FULL [8, 2048, 1024] f32 output of:
    BxW0 = einsum('bsd,bdr,do->bsro', x.astype(fp16), adapter_b, W0)
    out  = relu(mean(swapaxes(adapter_a,1,2)[:,None]*BxW0.reshape(b,s,d,r), -1))

Math refactor (verified exactly): with o = kk*128 + g*16 + mp,
    W_eff[dd, o] = adapter_b[dd, kk] * sum_rp adapter_a[rp, o] * W0[dd, (o%128)*8 + rp]
    out[b] = relu((x_fp16[b] @ W_eff[b]) / 8)
which is one [2048,1024] @ [1024,1024] matmul per batch -- data-parallel over
the batch dim: batch b runs on NeuronCore b (sharding_hint's layout).

Host does ONLY data placement (sharding/permutation/replication, no
arithmetic): X^T and W0^T tile-packing, a block-diagonal embedding of
adapter_a (A_sp), and 16x replication of adapter_b (B_bc).

v3 schedule (from the v2 trace: first matmul at 12.1us, ~7.5us of PE gaps
in the build phase, 6.2us store-drain tail; aggregate DMA ~370 GB/s, one
HWDGE queue can't beat ~150 GB/s on 2KB rows -- per-row descriptor cost --
and each DIRECT2D trigger costs ~650ns on its sequencer):
  - Inputs ride THREE queues in parallel: sync (SP HWDGE) carries the W0
    chain then x6..x15; scalar (Act HWDGE) carries asp halves then x0..x5;
    SWDGE (gpsimd) carries bbc. All triggers are emitted up front, so each
    queue FIFO is its own priority list and the W0 chain is never blocked
    behind x-tile data.
  - Cast work is split by engine: DVE does W0-chunk casts + the BTT
    (W_eff = b * C) multiplies, in FIFO order [btt(t-1), cast(t)] (the
    build phase is DMA-bound, so a cast-ahead would head-of-line-block
    the btt behind the next chunk's DMA); the Act engine casts asp and
    x0/x1; x2..x15 casts ride DVE, paired after btts whose data has
    landed, so they never block the chain.
  - 3 warm s-tiles (PSUM: 6 banks + 2 for the C chain) consume W_eff
    chunks as they are built; remaining 13 tiles run back-to-back after,
    4 in flight via the freed C banks.
  - Stores: even tiles -> Act HWDGE, odd tiles -> SWDGE (last SWDGE use
    at tile 13 so its final DRAIN overlaps compute); tiles 14/15 are
    split into 64-row halves across sync+scalar HWDGE queues in parallel
    so the tail after the last matmul is ~1.5us, not 6us.
"""

from contextlib import ExitStack

import numpy as np

S, D, R = 2048, 1024, 8
NT = D // 128
NS = S // 128
WARM = [0, 1, 2]
N_CORES = 8

_compiled = None


def _build_kernel():
    import concourse.bass as bass
    import concourse.tile as tile
    from concourse import bacc, mybir

    F32 = mybir.dt.float32
    F16 = mybir.dt.float16

    nc = bacc.Bacc(
        "TRN2", target_bir_lowering=False, debug=False, num_devices=N_CORES
    )

    x_d = nc.dram_tensor("xtp", [NS, 128, D], F32, kind="ExternalInput").ap()
    w0_d = nc.dram_tensor("w0tp", [NT, 128, D], F32, kind="ExternalInput").ap()
    asp_d = nc.dram_tensor("asp", [128, NT * 128], F32, kind="ExternalInput").ap()
    bbc_d = nc.dram_tensor("bbc", [128, NT * R], F32, kind="ExternalInput").ap()
    out_d = nc.dram_tensor("out", [S, D], F32, kind="ExternalOutput").ap()

    with tile.TileContext(nc) as tc, ExitStack() as ctx:
        pool = lambda name, bufs, **kw: ctx.enter_context(
            tc.tile_pool(name=name, bufs=bufs, **kw)
        )
        const_p = pool("const", 1)
        w0stage_p = pool("w0stage", 8)
        w0t_p = pool("w0t", 1)
        weff_p = pool("weff", 1)
        xstage_p = pool("xstage", 12)
        xth_p = pool("xth", 6)
        outst_p = pool("outst", 5)
        pmm_w = pool("pmmw", 6, space="PSUM")
        pmm_c = pool("pmmc", 2, space="PSUM")

        import concourse.mybir as mybir_mod

        Copy = mybir_mod.ActivationFunctionType.Copy
        Relu = mybir_mod.ActivationFunctionType.Relu

        # ---------- input DMA triggers, all up front ----------
        # ONE input queue (sync): a single HWDGE queue saturates the
        # ~370 GB/s core HBM share, and parallel queues only SPLIT it
        # (v3a measured the W0 chain at half rate when x rode a second
        # queue). The FIFO order is the priority list. Only bbc rides
        # SWDGE: it is small, off the critical path until btt(0), and
        # SWDGE is otherwise idle until the odd-tile stores.
        bbc = const_p.tile([128, NT * 128], F32, tag="bbc")
        abr = const_p.tile([128, NT * R], F32, tag="abr")
        nc.gpsimd.dma_start(abr[:], bbc_d[:])

        asp_st = const_p.tile([128, NT * 128], F32, tag="asp_st")
        w0s_tiles = []
        xs_tiles = {}

        def w0_dma(t):
            w0s = w0stage_p.tile([128, D], F32, tag="w0s", name=f"w0s{t}")
            nc.sync.dma_start(w0s[:], w0_d[t])
            w0s_tiles.append(w0s)

        def x_dma(s):
            xs = xstage_p.tile([128, D], F32, tag="xs", name=f"xs{s}")
            nc.sync.dma_start(xs[:], x_d[s])
            xs_tiles[s] = xs

        w0s0 = w0stage_p.tile([128, D], F32, tag="w0s", name="w0s0")
        nc.scalar.dma_start(asp_st[:, 0:512], asp_d[:, 0:512])
        nc.scalar.dma_start(asp_st[:, 512:1024], asp_d[:, 512:1024])
        nc.sync.dma_start(w0s0[:, 0:512], w0_d[0][:, 0:512])
        nc.sync.dma_start(w0s0[:, 512:1024], w0_d[0][:, 512:1024])
        w0s_tiles.append(w0s0)
        x_dma(0)
        w0_dma(1)
        x_dma(1)
        w0_dma(2)
        x_dma(2)
        w0_dma(3)
        w0_dma(4)
        w0_dma(5)
        w0_dma(6)
        w0s7 = w0stage_p.tile([128, D], F32, tag="w0s", name="w0s7")
        nc.sync.dma_start(w0s7[:, 0:512], w0_d[7][:, 0:512])
        nc.sync.dma_start(w0s7[:, 512:1024], w0_d[7][:, 512:1024])
        w0s_tiles.append(w0s7)
        # x3.. feed MAIN tiles (first consumer ~W_eff-complete time), and
        # every PSUM bank is held until btt(7): w0_7 gates the main phase,
        # so all non-warm x tiles go after the W0 chain
        for s in range(3, NS):
            x_dma(s)

        # ---------- SBUF fp16 targets ----------
        w0t = w0t_p.tile([128, NT * 1024], F16, tag="w0t")
        weff = weff_p.tile([128, NT * 1024], F16, tag="weff")
        asp_h = const_p.tile([128, NT * 128], F16, tag="asp_h")

        xth_tiles = {}

        def x_cast(s, eng="vector"):
            xth = xth_p.tile([128, D], F16, tag="xth", name=f"xth{s}")
            if eng == "vector":
                nc.vector.tensor_copy(xth[:], xs_tiles[s][:])
            else:
                nc.scalar.activation(xth[:], xs_tiles[s][:], Copy)
            xth_tiles[s] = xth

        def w0_cast(t):
            nc.vector.tensor_copy(
                w0t[:, t * 1024 : (t + 1) * 1024], w0s_tiles[t][:]
            )

        # ---------- per-chunk builders ----------
        pcs_tiles = {}

        def c_alloc(t):
            pcs_tiles[t] = [
                pmm_c.tile([128, 512], F32, tag="pmmc", name=f"pc{t}_{i}")
                for i in range(2)
            ]

        def c_mms(t, gs=range(NT)):
            if t not in pcs_tiles:
                c_alloc(t)
            pcs = pcs_tiles[t]
            for g in gs:
                nc.tensor.matmul(
                    pcs[g // 4][:, (g % 4) * 128 : (g % 4 + 1) * 128],
                    lhsT=w0t[:, t * 1024 + g * 128 : t * 1024 + (g + 1) * 128],
                    rhs=asp_h[:, g * 128 : (g + 1) * 128],
                    start=True,
                    stop=True,
                )

        def btt(t):
            pcs = pcs_tiles[t]
            wv = weff[:, t * 1024 : (t + 1) * 1024].rearrange(
                "p (kk g mp) -> p kk g mp", kk=8, g=NT, mp=16
            )
            for half in range(2):
                wvh = wv[:, :, half * 4 : (half + 1) * 4, :]
                bv = bbc[:, t * 128 : (t + 1) * 128].rearrange(
                    "p (kk mp) -> p kk mp", kk=8
                )[:, :, None, :].broadcast_to([128, 8, 4, 16])
                pv = pcs[half].rearrange("p (g kk mp) -> p kk g mp", g=4, kk=8)
                nc.vector.tensor_tensor(
                    out=wvh, in0=pv, in1=bv, op=mybir_mod.AluOpType.mult
                )

        def mm_pair(po, xth, c):
            for h in range(2):
                nc.tensor.matmul(
                    po[h][:],
                    lhsT=xth[:, c * 128 : (c + 1) * 128],
                    rhs=weff[:, c * 1024 + h * 512 : c * 1024 + (h + 1) * 512],
                    start=(c == 0),
                    stop=(c == NT - 1),
                )

        def evac(s, po):
            outst = outst_p.tile([128, D], F32, tag="outst", name=f"outst{s}")
            nc.scalar.activation(outst[:, 0:512], po[0][:], Relu, scale=0.125)
            nc.vector.tensor_scalar(
                out=outst[:, 512:1024],
                in0=po[1][:],
                scalar1=0.125,
                scalar2=0.0,
                op0=mybir_mod.AluOpType.mult,
                op1=mybir_mod.AluOpType.max,
            )
            return outst

        def store_full(s, outst, eng):
            eng.dma_start(out_d[s * 128 : (s + 1) * 128, :], outst[:])

        def store_split(s, outst, three_way=False):
            # row-split across queues in parallel: rows are 4KB-contiguous
            # DRAM, and per-queue store rate is ~131 GB/s, so splitting is
            # a direct tail win
            if three_way:
                nc.sync.dma_start(
                    out_d[s * 128 : s * 128 + 43, :], outst[0:43, :]
                )
                nc.scalar.dma_start(
                    out_d[s * 128 + 43 : s * 128 + 86, :], outst[43:86, :]
                )
                nc.gpsimd.dma_start(
                    out_d[s * 128 + 86 : (s + 1) * 128, :], outst[86:128, :]
                )
            else:
                for i, eng in enumerate((nc.sync, nc.scalar, nc.sync, nc.scalar)):
                    r0, r1 = i * 32, (i + 1) * 32
                    eng.dma_start(
                        out_d[s * 128 + r0 : s * 128 + r1, :], outst[r0:r1, :]
                    )

        # ---------- warm phase: W_eff build + 3 warm s-tiles ----------
        po_warm = {
            s: [
                pmm_w.tile([128, 512], F32, tag="pmmw", name=f"po{s}_{i}")
                for i in range(2)
            ]
            for s in WARM
        }

        # bbc = 16x on-device broadcast of raw adapter_b (saves 480KB of
        # critical head-stream DMA; host used to ship the replicated form)
        nc.vector.tensor_copy(
            bbc.rearrange("p (t kk mp) -> p t kk mp", t=NT, kk=R, mp=16),
            abr.rearrange("p (t kk) -> p t kk", t=NT)[:, :, :, None]
            .broadcast_to([128, NT, R, 16]),
        )

        # chunk 0 at half granularity: each half's casts + C matmuls gate
        # only on that half's DMA
        nc.vector.tensor_copy(w0t[:, 0:512], w0s_tiles[0][:, 0:512])
        nc.scalar.activation(asp_h[:, 0:512], asp_st[:, 0:512], Copy)
        c_mms(0, range(4))
        nc.vector.tensor_copy(w0t[:, 512:1024], w0s_tiles[0][:, 512:1024])
        nc.scalar.activation(asp_h[:, 512:1024], asp_st[:, 512:1024], Copy)
        c_mms(0, range(4, NT))
        btt(0)
        # x0/x1 casts on Act: they are the first pair enablers and the
        # DVE FIFO is busy with the chunk chain
        x_cast(0, eng="scalar")
        x_cast(1, eng="scalar")

        # DVE x-cast pairing: after btt(t), cast the x tile that has
        # surely landed by then, so a cast can never head-of-line-block
        # the next btt in the DVE FIFO
        xc_after_btt = {3: 2, 5: 3}

        cursor = {s: 0 for s in WARM}
        joined = {0: 1, 1: 1, 2: 4}  # iteration at which tile s starts pairs

        for t in range(1, NT):
            if t == NT - 1:
                # half-granular chain for the last chunk: btt(7) gates the
                # whole main phase (all PSUM banks are held until then)
                nc.vector.tensor_copy(
                    w0t[:, t * 1024 : t * 1024 + 512],
                    w0s_tiles[t][:, 0:512],
                )
                c_mms(t, range(4))
                nc.vector.tensor_copy(
                    w0t[:, t * 1024 + 512 : (t + 1) * 1024],
                    w0s_tiles[t][:, 512:1024],
                )
                c_mms(t, range(4, NT))
            else:
                w0_cast(t)
                c_mms(t)
            btt(t)
            if t in xc_after_btt:
                x_cast(xc_after_btt[t])
            # warm pairs: consume built chunks, up to 2 per tile per
            # iteration, at most 4 per iteration so a stalled pair can
            # never starve the next C(t) for long
            emitted = 0
            for s in WARM:
                if t < joined[s]:
                    continue
                tile_emitted = 0
                while cursor[s] < t and emitted < 4 and tile_emitted < 2:
                    mm_pair(po_warm[s], xth_tiles[s], cursor[s])
                    cursor[s] += 1
                    emitted += 1
                    tile_emitted += 1
        for s in WARM:
            while cursor[s] < NT:
                mm_pair(po_warm[s], xth_tiles[s], cursor[s])
                cursor[s] += 1

        warm_outst = {s: evac(s, po_warm[s]) for s in WARM}
        store_full(0, warm_outst[0], nc.scalar)
        store_full(1, warm_outst[1], nc.gpsimd)
        store_full(2, warm_outst[2], nc.scalar)

        # ---------- main phase: remaining 13 s-tiles ----------
        for s in range(len(WARM), NS):
            for tgt in (s, s + 1, s + 2):
                if tgt < NS and tgt not in xth_tiles:
                    x_cast(tgt)
            mpool = pmm_c if (s - len(WARM)) % 4 == 3 else pmm_w
            tg = "pmmc" if mpool is pmm_c else "pmmw"
            po = [
                mpool.tile([128, 512], F32, tag=tg, name=f"po{s}_{i}")
                for i in range(2)
            ]
            for c in range(NT):
                mm_pair(po, xth_tiles[s], c)
            outst = evac(s, po)
            if s >= NS - 2:
                # HWDGE queues only: a SWDGE store this late adds a ~7us
                # final DRAIN on the GpSimd sequencer (measured v3c)
                store_split(s, outst)
            elif s % 2 == 1:
                store_full(s, outst, nc.gpsimd)
            else:
                store_full(s, outst, nc.scalar)

    nc.compile()
    return nc


def _pack_inputs(x_b, adapter_b_b, adapter_a_b, W0):
    """Pure data placement (permutation / replication / zero-padding)."""
    xtp = np.ascontiguousarray(
        x_b.reshape(NS, 128, NT, 128).transpose(0, 3, 2, 1).reshape(NS, 128, D),
        np.float32,
    )
    w0tp = np.ascontiguousarray(
        W0.reshape(NT, 128, NT, 128).transpose(0, 3, 2, 1).reshape(NT, 128, D),
        np.float32,
    )
    asp = np.zeros((NT, 128, 128), np.float32)
    aa = adapter_a_b
    for g in range(NT):
        for mp in range(16):
            for rp in range(R):
                asp[g, mp * 8 + rp, np.arange(8) * 16 + mp] = aa[
                    rp, np.arange(8) * 128 + g * 16 + mp
                ]
    asp = np.ascontiguousarray(asp.transpose(1, 0, 2).reshape(128, NT * 128))
    # raw adapter_b, permuted to [dd, (t, kk)]; the kernel broadcasts the
    # 16x mp replication on-device
    bbc = np.ascontiguousarray(
        adapter_b_b.reshape(NT, 128, R).transpose(1, 0, 2).reshape(128, NT * R),
        np.float32,
    )
    return {"xtp": xtp, "w0tp": w0tp, "asp": asp, "bbc": bbc}


def kernel(x, adapter_b, adapter_a, W0):
    global _compiled
    x = np.asarray(x, np.float32)
    adapter_b = np.asarray(adapter_b, np.float32)
    adapter_a = np.asarray(adapter_a, np.float32)
    W0 = np.asarray(W0, np.float32)
    B = x.shape[0]
    assert B == N_CORES and x.shape == (B, S, D)

    if _compiled is None:
        _compiled = _build_kernel()

    from concourse.bass_utils import run_bass_kernel_spmd

    in_maps = [
        _pack_inputs(x[b], adapter_b[b], adapter_a[b], W0) for b in range(B)
    ]
    res = run_bass_kernel_spmd(_compiled, in_maps, list(range(N_CORES)))
    out = np.stack([res.results[b]["out"] for b in range(B)]).astype(np.float32)
    return out
